# revision 21
# baseline (speedup 1.0000x reference)
"""Binary 3x3 conv (sign(x) (*) sign(w)) + eval-mode BatchNorm for Trainium2.

Strategy
--------
Data-parallel over batch: 32 images -> 4 per NeuronCore x 8 cores. Conv
weights / BN params are replicated.

Per core, per image, the 3x3 stride-1 pad-1 conv is computed as 9 shifted
matmuls accumulating in PSUM. The activation image is kept in SBUF in a
zero-padded layout ([58 rows x 58 cols] per channel, plus one leading zero
guard row) so every kernel-tap shift is a contiguous window of the flat
padded buffer; the zero pad columns/rows provide the conv zero-padding for
free (including the row-wrap reads, which land on pad columns).

Both operands are exactly +-1 (or 0 at exact zeros, which `Sign` preserves),
so the matmul is exact in fp8e4m3 with fp32 PSUM accumulation (all partial
sums are integers |s| <= 2304 < 2^24).  fp8 enables DoubleRow perf mode:
the full Cin=256 contraction runs in one matmul pass (2 rows/cell), at 0.5
cycles per output column -- 2x the bf16 rate.

Pipeline per (image, cout-half): 7 PSUM bands of 8 output rows (464 fp32,
one bank each), weight-stationary inner loop (each of the 9 taps is loaded
once and swept over all 7 bands), then a fused BN affine (per-partition
scale+bias on VectorE) on the PSUM->SBUF evacuation, and a contiguous DMA
to the output.

The only host-side math is: dtype cast of x to bf16 (sign-lossless: bf16
shares fp32's exponent range), sign+pack of the small weight tensor, and
folding BN params into per-channel scale/bias vectors.
"""

import numpy as np
from contextlib import ExitStack

import ml_dtypes

import concourse.bass as bass  # noqa: F401  (import keeps bass registered)
import concourse.mybir as mybir
import concourse.tile as tile
from concourse import bacc
from concourse.bass_utils import run_bass_kernel_spmd

# Problem shapes (hardcoded per contract).
N, CIN, H, W = 32, 256, 56, 56
COUT = 256
N_CORES = 8
IMGS = N // N_CORES          # 4 images per core
PW = W + 2                   # padded row width: 58
ROWS_PER_BAND = 8
NBANDS = H // ROWS_PER_BAND  # 7
BAND_N = ROWS_PER_BAND * PW  # 464 fp32 <= 512 (one PSUM bank)
OUT_BAND = ROWS_PER_BAND * W  # 448
# Banded activation layout: the padded image is stored as 7 band-blocks of 10
# padded rows (8 output rows + halo), both cin-chunks adjacent per block, so
# each matmul's byte footprint is confined to its own block (precise,
# band-granular RAW/WAR tracking in Tile) and the DoubleRow rhs stays a 3D
# [K, 2, 464] AP. Halo rows are duplicated across neighboring blocks.
XB = 608                     # per chunk-block: 16 guard + 10*58 rows + 12 tail
BLK = 2 * XB                 # block stride (both chunks); 608 % 16 == 0 (DR rule)
XPLEN = NBANDS * BLK         # 8512 bytes/partition
BN_EPS = 1e-5

USE_FP8 = True


def emit(ctx, tc, x, w, bn, y, use_fp8=USE_FP8, imgs=IMGS):
    """Emit the per-core program.

    x:  [imgs, 256, 3136] bf16   (input activations, one shard)
    w:  [128, 9, 2, 2, 128]      (binarized weights: [cin_p, tap, cin_hi, cout_hi, cout_lo])
    bn: [2, 2, 128] f32          ([scale/bias, cout_hi, cout_lo])
    y:  [imgs, 256, 3136] f32
    """
    nc = tc.nc
    f32 = mybir.dt.float32
    dt_in = mybir.dt.float8e4 if use_fp8 else mybir.dt.bfloat16
    DR = mybir.MatmulPerfMode.DoubleRow

    wp = ctx.enter_context(tc.tile_pool(name="wp", bufs=1))
    bnp = ctx.enter_context(tc.tile_pool(name="bnp", bufs=1))
    # bufs=1 is deliberate: image k+1's DMA then waits for image k's sign
    # reads (WAR on the single slot), which keeps the startup window's HBM
    # bandwidth dedicated to image 0 instead of fair-shared across all
    # prefetches. Steady-state has a ~25us window per image, so the
    # serialization never surfaces.
    xinp = ctx.enter_context(tc.tile_pool(name="xinp", bufs=1))
    xpp = ctx.enter_context(tc.tile_pool(name="xpp", bufs=1))
    psp = ctx.enter_context(tc.tile_pool(name="psp", bufs=8, space="PSUM"))
    obp = ctx.enter_context(tc.tile_pool(name="obp", bufs=4))

    w_sb = wp.tile([128, 9, 2, 2, 128], dt_in)
    bn_sb = bnp.tile([128, 2, 2], f32)  # [cout_lo(part), scale/bias, cout_hi]

    # Three persistent padded-activation buffers rotating across images.
    # Three (not two) so an image's sign never chains behind matmuls still
    # reading a buffer two images back. Only border/guard cells are zeroed,
    # and only once: sign rewrites the data rows per image, everything else
    # stays zero.
    xpads = [
        xpp.tile([128, XPLEN], dt_in, name=f"xpad{i}", tag=f"xpad{i}")
        for i in range(3)
    ]
    # Dedicated zeroed tile for the PE warmup, memset first: the warmup
    # matmuls then gate on nothing else and start as soon as possible.
    warm_sb = wp.tile([128, 256], dt_in, name="warm_sb", tag="warm_sb")
    # GpSimd is the first engine out of the preamble (~6us); memset there so
    # the warmup matmuls start as early as possible.
    nc.gpsimd.memset(warm_sb[:], 0)
    # 1-element dummy sign: forces the Sign ACT_TABLE_LOAD to the front of
    # ScalarE's queue (no DMA deps), so the real signs aren't serialized
    # behind a late table load.
    nc.scalar.sign(warm_sb[:, 254:255], warm_sb[:, 254:255])
    for t in xpads:
        xv = t[:].rearrange("p (k t) -> p k t", t=XB)  # [128, 14 chunk-blocks, 608]
        # per chunk-block guard prefix + left pad col of row 0
        nc.vector.memset(xv[:, :, 0:17], 0)
        # right/left pad col pairs between consecutive rows 0..8
        nc.vector.memset(
            xv[:, :, 73 : 73 + 9 * PW].rearrange("p k (r t) -> p k r t", t=PW)[
                :, :, :, 0:2
            ],
            0,
        )
        # row 9 right pad col + block tail
        nc.vector.memset(xv[:, :, 595:XB], 0)
        # block 0 holds the top conv-pad row; block 6 the bottom one
        nc.vector.memset(xv[:, 0:2, 16:74], 0)
        nc.vector.memset(xv[:, 12:14, 538:596], 0)

    # Warm up the PE clock (HAM) during the startup DMA/sign window with
    # matmuls on already-zeroed SBUF (no DMA dependency); results go to a
    # scratch slot of the PSUM pool and are never read. ~6us of activity
    # flips the clock gate to 8/8 right before the real stream begins.
    if use_fp8:
        wm = psp.tile([128, BAND_N], f32, name="wm", tag="ps")
        for k in range(37):
            nc.tensor.matmul(
                wm[:, 0:128],
                warm_sb[:, 0:128],
                warm_sb[:, 128:256],
                start=True,
                stop=True,
            )

    def emit_sign(xi, xp, b, c):
        """sign() band-block b's data rows of chunk c into the padded buffer."""
        d0 = max(0, 8 * b - 1)       # first data row the block needs
        d1 = min(H, 8 * b + 9)       # one past the last
        r0 = d0 + 1 - 8 * b          # its row index within the block
        dst = (
            xp[:, (2 * b + c) * XB + 16 : (2 * b + c) * XB + 16 + 580]
            .rearrange("p (r t) -> p r t", t=PW)[:, r0 : r0 + (d1 - d0), 1:57]
        )
        src = xi[:, c, d0 * W : d1 * W].rearrange("p (a b) -> p a b", b=W)
        nc.scalar.sign(dst, src)

    def emit_mm(pss, xp, co, s, b, start, stop, c=None):
        dh, dw = divmod(s, 3)
        oi = 16 + dh * PW + dw - 1   # tap offset within a chunk-block
        if c is None:
            # N = 455: the 456th position (last row's pad col) is garbage,
            # so don't stream it.
            rhs = xp[:, b * BLK : (b + 1) * BLK].rearrange(
                "p (c k) -> p c k", c=2
            )[:, :, oi : oi + BAND_N - 1]
            nc.tensor.matmul(
                pss[b][:, 0 : BAND_N - 1],
                w_sb[:, s, :, co],
                rhs,
                start=start,
                stop=stop,
                perf_mode=DR,
            )
        else:
            nc.tensor.matmul(
                pss[b][:, 0 : BAND_N - 1],
                w_sb[:, s, c, co],
                xp[:, b * BLK + c * XB + oi : b * BLK + c * XB + oi + BAND_N - 1],
                start=start,
                stop=stop,
            )

    def emit_copy_out(img, co, pss, b, alternate=False):
        yv = y[img].rearrange("(t p) q -> t p q", p=128)[co]
        ob = obp.tile([128, OUT_BAND], f32, name="ob", tag="ob")
        psv = pss[b][:].rearrange("p (r q) -> p r q", q=PW)[:, :, 1:57]
        obv = ob[:].rearrange("p (r q) -> p r q", q=W)
        # Keep ScalarE free for the next image's signs: copies go to DVE,
        # except the final group's, which alternate DVE/ACT so the tail
        # drains on two engines.
        if not alternate or b % 2 == 0:
            nc.vector.tensor_scalar(
                obv,
                psv,
                bn_sb[:, 0, co : co + 1],
                bn_sb[:, 1, co : co + 1],
                op0=mybir.AluOpType.mult,
                op1=mybir.AluOpType.add,
            )
        else:
            nc.scalar.activation(
                obv,
                psv,
                mybir.ActivationFunctionType.Identity,
                bias=bn_sb[:, 1, co : co + 1],
                scale=bn_sb[:, 0, co : co + 1],
            )
        nc.sync.dma_start(yv[:, b * OUT_BAND : (b + 1) * OUT_BAND], ob[:])

    # Input strips: ascending row blocks sized so sign block b only waits on
    # the strips covering its rows -- the first sign (and so the first
    # matmul) starts after ~0.3MB instead of the whole 1.6MB image.
    # strip b ends exactly where sign block b's rows end (8b+9), so sign b
    # chains on strips 0..b with no over-wait
    STRIPS = [(0, 9), (9, 17), (17, 25), (25, 33), (33, 41), (41, 49), (49, 56)]

    for img in range(imgs):
        xi = xinp.tile([128, 2, H * W], mybir.dt.bfloat16)
        xsrc = x[img].rearrange("(c p) q -> c p q", p=128)
        xp = xpads[img % 3]
        for lo, hi in STRIPS:
            for c in range(2):
                # GpSimd queue: input strips must not sit behind the output
                # DMAs in Sync's FIFO, or the next image's strips would only
                # issue once this image's last output drains.
                nc.gpsimd.dma_start(
                    xi[:, c, lo * W : hi * W], xsrc[c][:, lo * W : hi * W]
                )
            if img == 0 and lo == 0:
                # Weights tap-by-tap: band-outer order consumes tap s only
                # ~s*0.2us into the stream, so the first matmul gates on a
                # 65KB slice instead of the whole 590KB tensor.
                for s in range(9):
                    nc.sync.dma_start(w_sb[:, s], w[:, s])
                nc.sync.dma_start(bn_sb[:], bn.rearrange("k c p -> p k c"))
        # Per-block signs: band b's matmuls only wait for its own blocks.
        for b in range(NBANDS):
            for c in range(2):
                emit_sign(xi, xp, b, c)

        for co in range(2):
            last = img == imgs - 1 and co == 1
            if last and use_fp8:
                # Final group: split band 6 into 6+2 rows so the drain after
                # the very last matmul is a 2-row copy+DMA, not 8 rows.
                yv = y[img].rearrange("(t p) q -> t p q", p=128)[co]
                pss = [
                    psp.tile([128, BAND_N], f32, name="ps", tag="ps")
                    for _ in range(6)
                ]
                for b in range(6):
                    for s in range(9):
                        emit_mm(pss, xp, co, s, b, s == 0, s == 8)
                    emit_copy_out(img, co, pss, b, alternate=True)
                base = 6 * BLK
                for r_lo, nr, on_dve in [(0, 6, True), (6, 2, False)]:
                    ps6 = psp.tile([128, nr * PW], f32, name="ps6", tag="ps")
                    for s in range(9):
                        dh, dw = divmod(s, 3)
                        oi = 16 + (dh + r_lo) * PW + dw - 1
                        rhs = xp[:, base : base + BLK].rearrange(
                            "p (c k) -> p c k", c=2
                        )[:, :, oi : oi + nr * PW - 1]
                        nc.tensor.matmul(
                            ps6[:, 0 : nr * PW - 1], w_sb[:, s, :, co], rhs,
                            start=s == 0, stop=s == 8, perf_mode=DR,
                        )
                    ob = obp.tile([128, nr * W], f32, name="ob6", tag="ob")
                    psv = ps6[:].rearrange("p (r q) -> p r q", q=PW)[:, :, 0:56]
                    obv = ob[:].rearrange("p (r q) -> p r q", q=W)
                    if on_dve:
                        nc.vector.tensor_scalar(
                            obv, psv,
                            bn_sb[:, 0, co : co + 1], bn_sb[:, 1, co : co + 1],
                            op0=mybir.AluOpType.mult, op1=mybir.AluOpType.add,
                        )
                    else:
                        nc.scalar.activation(
                            obv, psv, mybir.ActivationFunctionType.Identity,
                            bias=bn_sb[:, 1, co : co + 1],
                            scale=bn_sb[:, 0, co : co + 1],
                        )
                    o0 = (48 + r_lo) * W
                    nc.sync.dma_start(yv[:, o0 : o0 + nr * W], ob[:])
                continue
            pss = [
                psp.tile([128, BAND_N], f32, name="ps", tag="ps")
                for _ in range(NBANDS)
            ]
            # Band-outer everywhere: band b starts as soon as its sign
            # blocks land, its PSUM evacuation fires right after its 9th
            # tap, so copies/DMAs spread evenly instead of bunching at
            # group boundaries (and the tail drain is one band, not seven).
            for b in range(NBANDS):
                if use_fp8:
                    for s in range(9):
                        emit_mm(pss, xp, co, s, b, s == 0, s == 8)
                else:
                    for s in range(9):
                        for c in range(2):
                            emit_mm(
                                pss, xp, co, s, b,
                                s == 0 and c == 0, s == 8 and c == 1, c=c,
                            )
                emit_copy_out(img, co, pss, b, alternate=last)


_BUILT = {}


def _get_nc(use_fp8=USE_FP8, imgs=IMGS):
    key = (use_fp8, imgs)
    if key not in _BUILT:
        nc = bacc.Bacc(
            "TRN2", target_bir_lowering=False, debug=False, num_devices=N_CORES
        )
        dt_in = mybir.dt.float8e4 if use_fp8 else mybir.dt.bfloat16
        x_d = nc.dram_tensor(
            "x", [imgs, CIN, H * W], mybir.dt.bfloat16, kind="ExternalInput"
        )
        w_d = nc.dram_tensor("w", [128, 9, 2, 2, 128], dt_in, kind="ExternalInput")
        bn_d = nc.dram_tensor("bn", [2, 2, 128], mybir.dt.float32, kind="ExternalInput")
        y_d = nc.dram_tensor(
            "y", [imgs, COUT, H * W], mybir.dt.float32, kind="ExternalOutput"
        )
        with tile.TileContext(nc) as tc:
            with ExitStack() as ctx:
                emit(ctx, tc, x_d.ap(), w_d.ap(), bn_d.ap(), y_d.ap(), use_fp8, imgs)
        nc.compile()
        _BUILT[key] = nc
    return _BUILT[key]


def pack_weights(weight, use_fp8=USE_FP8):
    np_dt = ml_dtypes.float8_e4m3 if use_fp8 else ml_dtypes.bfloat16
    wb = np.sign(weight.astype(np.float32))
    # [cout, cin, kh, kw] -> [cin_lo(p), (kh kw), cin_hi, cout_hi, cout_lo(m)]
    wp = wb.reshape(2, 128, 2, 128, 3, 3).transpose(3, 4, 5, 2, 0, 1)
    return np.ascontiguousarray(wp.reshape(128, 9, 2, 2, 128)).astype(np_dt)


def pack_bn(gamma, beta, mean, var):
    inv = (gamma.astype(np.float32) / np.sqrt(var.astype(np.float32) + BN_EPS)).astype(
        np.float32
    )
    add = (beta.astype(np.float32) - mean.astype(np.float32) * inv).astype(np.float32)
    return np.ascontiguousarray(
        np.stack([inv.reshape(2, 128), add.reshape(2, 128)])
    ).astype(np.float32)


def kernel(**inputs):
    x = np.asarray(inputs["x"], dtype=np.float32)
    weight = np.asarray(inputs["weight"], dtype=np.float32)
    gamma = np.asarray(inputs["gamma"], dtype=np.float32)
    beta = np.asarray(inputs["beta"], dtype=np.float32)
    mean = np.asarray(inputs["running_mean"], dtype=np.float32)
    var = np.asarray(inputs["running_var"], dtype=np.float32)

    nc = _get_nc(USE_FP8)
    wp = pack_weights(weight, USE_FP8)
    bn = pack_bn(gamma, beta, mean, var)
    # bf16 cast is sign-exact for anything a float32 normal draw can produce
    # (bf16 keeps fp32's exponent range).
    xb = np.ascontiguousarray(x.reshape(N, CIN, H * W)).astype(ml_dtypes.bfloat16)

    in_maps = [
        {
            "x": np.ascontiguousarray(xb[core * IMGS : (core + 1) * IMGS]),
            "w": wp,
            "bn": bn,
        }
        for core in range(N_CORES)
    ]
    res = run_bass_kernel_spmd(nc, in_maps, core_ids=list(range(N_CORES)))
    y = np.empty((N, COUT, H, W), np.float32)
    for core in range(N_CORES):
        y[core * IMGS : (core + 1) * IMGS] = res.results[core]["y"].reshape(
            IMGS, COUT, H, W
        )
    return y
